# revision 1
# baseline (speedup 1.0000x reference)
"""Llama4 MoE experts kernel for 8 Trainium2 NeuronCores.

Expert-parallel: tokens are pre-sorted per expert (8192 tokens = 8 experts
x 1024 tokens), so core e gets expert e's tokens + weights and computes
   out_e = (up_e * silu(gate_e)) @ W2_e,   [gate_e|up_e] = x_e @ W1_e
entirely locally (no collectives). Matmuls run in bf16 with fp32 PSUM
accumulation; weights/activations are cast + laid out host-side so every
DMA is a long per-partition contiguous run and the PE streams at 1 row/cyc.
"""

import numpy as np
import ml_dtypes

E, T, H, F, P = 8, 1024, 2048, 4096, 128
KH, KF = H // P, F // P          # 16 k-blocks over H, 32 over F
CB = (2 * F) // P                # 64 column blocks of W1 (gate 0..31, up 32..63)
HB = H // 256                    # 8 output-column blocks of 256

_CACHE = {}


def _build():
    import concourse.bacc as bacc
    import concourse.tile as tile
    import concourse.mybir as mybir

    bf16 = mybir.dt.bfloat16
    f32 = mybir.dt.float32

    nc = bacc.Bacc("TRN2", target_bir_lowering=False, debug=False, num_devices=E)

    xt_d = nc.dram_tensor("xt", [P, KH, T], bf16, kind="ExternalInput").ap()
    w1_d = nc.dram_tensor("w1", [CB, P, KH, P], bf16, kind="ExternalInput").ap()
    w2_d = nc.dram_tensor("w2", [HB, P, KF, 256], bf16, kind="ExternalInput").ap()
    out_d = nc.dram_tensor("out", [T, H], f32, kind="ExternalOutput").ap()

    with tile.TileContext(nc) as tc:
        with (
            tc.tile_pool(name="resident", bufs=1) as res_pool,
            tc.tile_pool(name="w1pool", bufs=3) as w1_pool,
            tc.tile_pool(name="w2pool", bufs=2) as w2_pool,
            tc.tile_pool(name="tmppool", bufs=3) as tmp_pool,
            tc.tile_pool(name="outpool", bufs=4) as out_pool,
            tc.tile_pool(name="psg", bufs=2, space="PSUM") as psg_pool,
            tc.tile_pool(name="psu", bufs=2, space="PSUM") as psu_pool,
            tc.tile_pool(name="pso", bufs=4, space="PSUM") as pso_pool,
        ):
            xT = res_pool.tile([P, KH, T], bf16, name="xT")
            interT = res_pool.tile([P, KF, T], bf16, name="interT")

            # Phase 1: gate/up projections + SwiGLU -> interT (F on partitions)
            # DMA order matters for ramp-up: the first matmul chain needs
            # xT[:, 0] + w1g_0, so interleave the W1 i=0 tiles right after
            # the first xT block instead of queueing all of xT first.
            for i in range(KF):
                w1g = w1_pool.tile([P, KH, P], bf16, tag="w1g", name=f"w1g_{i}")
                w1u = w1_pool.tile([P, KH, P], bf16, tag="w1u", name=f"w1u_{i}")
                if i == 0:
                    nc.sync.dma_start(out=xT[:, 0, :], in_=xt_d[:, 0, :])
                nc.sync.dma_start(out=w1g[:], in_=w1_d[i])
                nc.sync.dma_start(out=w1u[:], in_=w1_d[KF + i])
                if i == 0:
                    for kb in range(1, KH):
                        nc.sync.dma_start(out=xT[:, kb, :], in_=xt_d[:, kb, :])
                for th in range(2):
                    ts_ = slice(th * 512, (th + 1) * 512)
                    pg = psg_pool.tile([P, 512], f32, tag="pg", name=f"pg_{i}_{th}")
                    pu = psu_pool.tile([P, 512], f32, tag="pu", name=f"pu_{i}_{th}")
                    for kb in range(KH):
                        nc.tensor.matmul(
                            pg[:], lhsT=w1g[:, kb, :], rhs=xT[:, kb, ts_],
                            start=(kb == 0), stop=(kb == KH - 1),
                        )
                    for kb in range(KH):
                        nc.tensor.matmul(
                            pu[:], lhsT=w1u[:, kb, :], rhs=xT[:, kb, ts_],
                            start=(kb == 0), stop=(kb == KH - 1),
                        )
                    sg = tmp_pool.tile([P, 512], f32, tag="sg", name=f"sg_{i}_{th}")
                    nc.scalar.activation(
                        sg[:], pg[:], mybir.ActivationFunctionType.Silu
                    )
                    nc.vector.tensor_mul(interT[:, i, ts_], sg[:], pu[:])

            # Phase 2: down projection, streaming W2 once
            for hb in range(HB):
                w2t = w2_pool.tile([P, KF, 256], bf16, tag="w2", name=f"w2_{hb}")
                nc.sync.dma_start(out=w2t[:], in_=w2_d[hb])
                for tb in range(T // P):
                    po = pso_pool.tile([P, 256], f32, tag="po", name=f"po_{hb}_{tb}")
                    for kb in range(KF):
                        nc.tensor.matmul(
                            po[:],
                            lhsT=interT[:, kb, tb * P:(tb + 1) * P],
                            rhs=w2t[:, kb, :],
                            start=(kb == 0), stop=(kb == KF - 1),
                        )
                    ob = out_pool.tile([P, 256], f32, tag="ob", name=f"ob_{hb}_{tb}")
                    nc.scalar.copy(ob[:], po[:])
                    nc.sync.dma_start(
                        out=out_d[tb * P:(tb + 1) * P, hb * 256:(hb + 1) * 256],
                        in_=ob[:],
                    )

    nc.compile()
    return nc


def _prep_inputs(hidden_states, gate_up_proj, down_proj):
    bf = ml_dtypes.bfloat16
    xr = np.asarray(hidden_states, np.float32).reshape(E, T, H)
    # xt[e, p, k, t] = x[e, t, k*128+p]
    xt = xr.transpose(0, 2, 1).reshape(E, KH, P, T).transpose(0, 2, 1, 3)
    xt = np.ascontiguousarray(xt).astype(bf)
    # w1b[e, c, p, k, j] = W1[e, k*128+p, c*128+j]
    w1b = np.asarray(gate_up_proj, np.float32).reshape(E, KH, P, CB, P)
    w1b = np.ascontiguousarray(w1b.transpose(0, 3, 2, 1, 4)).astype(bf)
    # w2b[e, hb, p, kb, j] = W2[e, kb*128+p, hb*256+j]
    w2b = np.asarray(down_proj, np.float32).reshape(E, KF, P, HB, 256)
    w2b = np.ascontiguousarray(w2b.transpose(0, 3, 2, 1, 4)).astype(bf)
    return [
        {"xt": np.ascontiguousarray(xt[e]),
         "w1": np.ascontiguousarray(w1b[e]),
         "w2": np.ascontiguousarray(w2b[e])}
        for e in range(E)
    ]


def run_spmd(in_maps, trace=False, trace_kwargs=None):
    from concourse.bass_utils import run_bass_kernel_spmd
    from concourse.bass_interp import get_hw_module

    if "nc" not in _CACHE:
        _CACHE["nc"] = _build()
    nc = _CACHE["nc"]

    old_m = nc.m
    nc.m = get_hw_module(nc.m)
    try:
        res = run_bass_kernel_spmd(
            nc, in_maps, core_ids=list(range(E)),
            trace=trace, **(trace_kwargs or {}),
        )
    finally:
        nc.m = old_m
    return res


def kernel(hidden_states, gate_up_proj, down_proj):
    in_maps = _prep_inputs(hidden_states, gate_up_proj, down_proj)
    res = run_spmd(in_maps)
    out = np.concatenate([res.results[e]["out"] for e in range(E)], axis=0)
    return out.astype(np.float32)



# revision 2
# speedup vs baseline: 1.0009x; 1.0009x over previous
"""Llama4 MoE experts kernel for 8 Trainium2 NeuronCores.

Expert-parallel + level-1 Strassen on both GEMMs. Tokens are pre-sorted per
expert (8192 = 8 experts x 1024), so core e computes expert e locally:
   out_e = (up_e * silu(gate_e)) @ W2_e,  [gate_e|up_e] = x_e @ W1_e

Strassen halves each GEMM's (M, K, N) once -> 7/8 of the PE row-streaming
time. All weight-side block combos (W1, W2) and the x-side combos are
precomputed on the host for free; the inter-side combos for the down-proj
are built on the idle Vector/GpSimd engines while phase 1 runs. M-products
accumulate in 7 PSUM banks per set; Act copies + DVE/GpSimd adds form the
C-blocks, with SwiGLU fused into the phase-1 combine.

Phase-1 PE form: psum[f128, t512] += s_m[:,kb,:].T @ b_m[:,kb,:]
Phase-2 PE form: psum[h128, t512] += w2s_m[:,kf,:].T @ mv_m[:,kf,:]
Phase-2 output is [h, t]-transposed; host un-transposes.
"""

import numpy as np
import ml_dtypes

E, T, H, F, P = 8, 1024, 2048, 4096, 128
TH = T // 2            # 512 token half
KB1 = 8                # k-blocks per K-half in phase 1 (K=1024)
KB2 = 16               # k-blocks per K-half in phase 2 (F/2=2048)
FB = 32                # 128-wide f blocks per f-half (4096/128)
HB = 8                 # 128-wide h blocks per h-half (1024/128)

# chain order within a 7-product set: delivers combine-input products
# early so DVE/Act work spreads across the set instead of cramming at
# its end (which shrinks PSUM-bank reuse margins and stalls the PE).
# Set 0 instead follows x-quadrant DMA arrival order (plain-quadrant
# products first, combo-dependent ones last).
CHAIN = [3, 5, 1, 2, 6, 4, 7]
CHAIN0 = [2, 5, 1, 3, 6, 7, 4]

_CACHE = {}


def _build():
    import concourse.bacc as bacc
    import concourse.tile as tile
    import concourse.mybir as mybir

    bf16 = mybir.dt.bfloat16
    f32 = mybir.dt.float32
    Silu = mybir.ActivationFunctionType.Silu

    nc = bacc.Bacc("TRN2", target_bir_lowering=False, debug=False, num_devices=E)

    xq_d = nc.dram_tensor("xq", [4, P, KB1 * TH], bf16, kind="ExternalInput").ap()
    w1s_d = nc.dram_tensor("w1s", [7, FB, P, KB1, P], bf16, kind="ExternalInput").ap()
    w2s_d = nc.dram_tensor("w2s", [7, HB, P, KB2, P], bf16, kind="ExternalInput").ap()
    out_d = nc.dram_tensor("outT", [2 * HB, P, T], f32, kind="ExternalOutput").ap()

    with tile.TileContext(nc) as tc:
        with (
            tc.tile_pool(name="bmv", bufs=1) as bmv_pool,       # 7 x [P,8,512] bf16
            tc.tile_pool(name="mv2", bufs=1) as mv2_pool,       # 7 x [P,16,512] bf16
            tc.tile_pool(name="spool", bufs=4) as s_pool,       # w1s stream ring
            tc.tile_pool(name="w2pool", bufs=3) as w2_pool,     # w2s stream ring
            tc.tile_pool(name="tmp", bufs=8) as tmp_pool,       # [P,512] f32 ring
            tc.tile_pool(name="piece", bufs=2) as piece_pool,   # [P,512] bf16 ring
            tc.tile_pool(name="ps", bufs=8, space="PSUM") as ps_pool,
        ):
            # phase-1 moving operands: q11/q22 quadrants double as b2/b5;
            # the other 5 combos are built on DVE during the DMA window.
            # Flat 2D layout so whole-tile DVE ops get the packed fast path.
            bt = {m: bmv_pool.tile([P, KB1 * TH], bf16, name=f"b{m}") for m in range(1, 8)}
            # zero tile for PE warmup matmuls (keeps the clock ramped while
            # the head DMA streams in; idle gaps reset the PE power state)
            dx = bmv_pool.tile([P, TH], bf16, name="dx")
            # phase-2 moving operands (inter combos), filled during phase 1
            q11 = mv2_pool.tile([P, KB2, TH], bf16, name="q11")
            q22 = mv2_pool.tile([P, KB2, TH], bf16, name="q22")
            u1 = mv2_pool.tile([P, KB2, TH], bf16, name="u1")
            u2 = mv2_pool.tile([P, KB2, TH], bf16, name="u2")
            u5 = mv2_pool.tile([P, KB2, TH], bf16, name="u5")
            u6 = mv2_pool.tile([P, KB2, TH], bf16, name="u6")
            u7 = mv2_pool.tile([P, KB2, TH], bf16, name="u7")
            mv2 = {1: u1, 2: u2, 3: q11, 4: q22, 5: u5, 6: u6, 7: u7}

            def tmp(nm):
                return tmp_pool.tile([P, TH], f32, tag="t", name=nm)

            # ---------------- phase 1 ----------------
            # Head: 4 whole-tile x-quadrant DMAs on the scalar queue (so the
            # ring-gated s-tile stream on sync can't head-of-line block them).
            # q12 stages in bt[6], q21 in bt[4]; the 5 combos are built with
            # whole-tile DVE/GpSimd ops (two of them in-place) while the DMA
            # streams. The PE runs warmup matmuls on a zero tile meanwhile —
            # an idle PE drops its clock to the mid power state.
            nc.vector.memset(dx[:], 0)
            for dc in range(4):
                dpm = ps_pool.tile([P, TH], f32, tag="m", name=f"dummy_{dc}")
                for kb in range(KB1):
                    nc.tensor.matmul(
                        dpm[:], lhsT=dx[:, 0:P], rhs=dx[:],
                        start=(kb == 0), stop=(kb == KB1 - 1),
                    )

            def s0_dma(m):
                st = s_pool.tile([P, KB1, P], bf16, tag="s", name=f"s{m}_0")
                nc.sync.dma_start(out=st[:], in_=w1s_d[m - 1, 0])
                return st

            # x-quadrant stream split across BOTH DMA queues (sync carries
            # q22/q21 JIT-interleaved with s-tiles, scalar carries q11/q12)
            # so neither stream head-of-line blocks the other and both
            # halves of HBM bandwidth feed set 0. Combos are Vector-only
            # (GpSimd tensor ops are ~10x slower) in half-tile granularity
            # so set-0 chains can stream behind them via subtile deps.
            HF = KB1 * TH // 2
            s0 = {}
            nc.scalar.dma_start(out=bt[2][:], in_=xq_d[0])      # q11 (= b2)
            nc.sync.dma_start(out=bt[5][:], in_=xq_d[1])        # q22 (= b5)
            s0[2] = s0_dma(2)
            s0[5] = s0_dma(5)
            s0[1] = s0_dma(1)
            nc.scalar.dma_start(out=bt[6][:], in_=xq_d[2])      # q12 (staged)
            s0[3] = s0_dma(3)
            for h in range(2):                                  # b1 = q11+q22
                hs = slice(h * HF, (h + 1) * HF)
                nc.vector.tensor_add(bt[1][:, hs], bt[2][:, hs], bt[5][:, hs])
            nc.sync.dma_start(out=bt[4][:], in_=xq_d[3])        # q21 (staged)
            s0[6] = s0_dma(6)
            for h in range(2):                                  # b3 = q12-q22
                hs = slice(h * HF, (h + 1) * HF)
                nc.vector.tensor_sub(bt[3][:, hs], bt[6][:, hs], bt[5][:, hs])
            for h in range(2):                                  # b6 = q12+q11
                hs = slice(h * HF, (h + 1) * HF)
                nc.vector.tensor_add(bt[6][:, hs], bt[6][:, hs], bt[2][:, hs])
            s0[7] = s0_dma(7)
            for h in range(2):                                  # b7 = q21+q22
                hs = slice(h * HF, (h + 1) * HF)
                nc.vector.tensor_add(bt[7][:, hs], bt[4][:, hs], bt[5][:, hs])
            for h in range(2):                                  # b4 = q21-q11
                hs = slice(h * HF, (h + 1) * HF)
                nc.vector.tensor_sub(bt[4][:, hs], bt[4][:, hs], bt[2][:, hs])
            s0[4] = s0_dma(4)

            for kf in range(KB2):
                # pair (fb=kf, fb=kf+16) -> f-half1 block then f-half2 block
                for half, fb in ((0, kf), (1, 16 + kf)):
                    ms = {}
                    for m in (CHAIN0 if fb == 0 else CHAIN):
                        if fb == 0:
                            st = s0[m]
                        else:
                            st = s_pool.tile([P, KB1, P], bf16, tag="s", name=f"s{m}_{fb}")
                            nc.sync.dma_start(out=st[:], in_=w1s_d[m - 1, fb])
                        pm = ps_pool.tile([P, TH], f32, tag="m", name=f"p1m{m}_{fb}")
                        for kb in range(KB1):
                            nc.tensor.matmul(
                                pm[:], lhsT=st[:, kb, :],
                                rhs=bt[m][:, kb * TH:(kb + 1) * TH],
                                start=(kb == 0), stop=(kb == KB1 - 1),
                            )
                        ms[m] = pm
                    # combine: C-blocks from M1..M7, SwiGLU fused.
                    # th1 path first (gate C12 = M3+M5, up C22 = M1-M2+M3+M6) --
                    # ordering matters for tmp-ring reuse across engine queues.
                    a3 = tmp(f"a3_{fb}")
                    nc.scalar.copy(a3[:], ms[3][:])
                    c12 = tmp(f"c12_{fb}")
                    nc.vector.tensor_add(c12[:], a3[:], ms[5][:])
                    sg1 = tmp(f"sg1_{fb}")
                    nc.scalar.activation(sg1[:], c12[:], Silu)
                    a1 = tmp(f"a1_{fb}")
                    nc.scalar.copy(a1[:], ms[1][:])
                    t3 = tmp(f"t3_{fb}")
                    nc.vector.tensor_sub(t3[:], a1[:], ms[2][:])
                    t4 = tmp(f"t4_{fb}")
                    nc.vector.tensor_add(t4[:], a3[:], ms[6][:])
                    c22 = tmp(f"c22_{fb}")
                    nc.gpsimd.tensor_add(c22[:], t3[:], t4[:])
                    # th1 inter piece = silu(C12)*C22
                    if half == 0:
                        q21p = piece_pool.tile([P, TH], bf16, tag="p", name=f"q21p_{kf}")
                        nc.gpsimd.tensor_mul(q21p[:], sg1[:], c22[:])
                        held_q21 = q21p
                    else:
                        nc.gpsimd.tensor_mul(q22[:, kf, :], sg1[:], c22[:])
                    # th0 path (gate C11 = M1+M4-M5+M7, up C21 = M2+M4)
                    a4 = tmp(f"a4_{fb}")
                    nc.scalar.copy(a4[:], ms[4][:])
                    c21 = tmp(f"c21_{fb}")
                    nc.vector.tensor_add(c21[:], a4[:], ms[2][:])
                    t1 = tmp(f"t1_{fb}")
                    nc.gpsimd.tensor_add(t1[:], a1[:], a4[:])
                    t2 = tmp(f"t2_{fb}")
                    nc.vector.tensor_sub(t2[:], t1[:], ms[5][:])
                    c11 = tmp(f"c11_{fb}")
                    nc.vector.tensor_add(c11[:], t2[:], ms[7][:])
                    sg0 = tmp(f"sg0_{fb}")
                    nc.scalar.activation(sg0[:], c11[:], Silu)
                    # th0 inter piece = silu(C11)*C21
                    if half == 0:
                        nc.gpsimd.tensor_mul(q11[:, kf, :], sg0[:], c21[:])
                    else:
                        q12p = piece_pool.tile([P, TH], bf16, tag="p", name=f"q12p_{kf}")
                        nc.gpsimd.tensor_mul(q12p[:], sg0[:], c21[:])
                        # u combos for this kf
                        nc.vector.tensor_add(u1[:, kf, :], q11[:, kf, :], q22[:, kf, :])
                        nc.vector.tensor_add(u2[:, kf, :], held_q21[:], q22[:, kf, :])
                        nc.gpsimd.tensor_add(u5[:, kf, :], q11[:, kf, :], q12p[:])
                        nc.vector.tensor_sub(u6[:, kf, :], held_q21[:], q11[:, kf, :])
                        nc.gpsimd.tensor_sub(u7[:, kf, :], q12p[:], q22[:, kf, :])

            # ---------------- phase 2 ----------------
            for hb in range(HB):
                ms = {}
                for m in CHAIN:
                    wt = w2_pool.tile([P, KB2, P], bf16, tag="w2", name=f"w2_{m}_{hb}")
                    nc.sync.dma_start(out=wt[:], in_=w2s_d[m - 1, hb])
                    pm = ps_pool.tile([P, TH], f32, tag="m", name=f"p2m{m}_{hb}")
                    for kf in range(KB2):
                        nc.tensor.matmul(
                            pm[:], lhsT=wt[:, kf, :], rhs=mv2[m][:, kf, :],
                            start=(kf == 0), stop=(kf == KB2 - 1),
                        )
                    ms[m] = pm
                # combine to out tiles [h128, t512]
                a3 = tmp(f"pa3_{hb}")
                nc.scalar.copy(a3[:], ms[3][:])
                o12 = tmp(f"o12_{hb}")
                nc.vector.tensor_add(o12[:], a3[:], ms[5][:])
                nc.scalar.dma_start(out=out_d[HB + hb, :, 0:TH], in_=o12[:])
                a1 = tmp(f"pa1_{hb}")
                nc.scalar.copy(a1[:], ms[1][:])
                t3 = tmp(f"pt3_{hb}")
                nc.vector.tensor_sub(t3[:], a1[:], ms[2][:])
                t4 = tmp(f"pt4_{hb}")
                nc.vector.tensor_add(t4[:], a3[:], ms[6][:])
                o22 = tmp(f"po22_{hb}")
                nc.gpsimd.tensor_add(o22[:], t3[:], t4[:])
                nc.scalar.dma_start(out=out_d[HB + hb, :, TH:T], in_=o22[:])
                a4 = tmp(f"pa4_{hb}")
                nc.scalar.copy(a4[:], ms[4][:])
                o21 = tmp(f"po21_{hb}")
                nc.vector.tensor_add(o21[:], a4[:], ms[2][:])
                nc.scalar.dma_start(out=out_d[hb, :, TH:T], in_=o21[:])
                t1 = tmp(f"pt1_{hb}")
                nc.gpsimd.tensor_add(t1[:], a1[:], a4[:])
                t2 = tmp(f"pt2_{hb}")
                nc.vector.tensor_sub(t2[:], t1[:], ms[5][:])
                o11 = tmp(f"po11_{hb}")
                nc.vector.tensor_add(o11[:], t2[:], ms[7][:])
                nc.scalar.dma_start(out=out_d[hb, :, 0:TH], in_=o11[:])

    nc.compile()
    return nc


def _prep_inputs(hidden_states, gate_up_proj, down_proj):
    bf = ml_dtypes.bfloat16
    xr = np.asarray(hidden_states, np.float32).reshape(E, T, H)
    w1 = np.asarray(gate_up_proj, np.float32)
    w2 = np.asarray(down_proj, np.float32)
    maps = []
    for e in range(E):
        # x^T blocked: xt[p, k, t] = x[t, k*128+p]; ship the 4 Strassen
        # quadrants (combos are built on-device during the DMA window)
        xt = xr[e].T.reshape(H // P, P, T).transpose(1, 0, 2)  # [128, 16, 1024]
        xq = np.stack([
            xt[:, 0:8, 0:TH], xt[:, 8:16, TH:T],
            xt[:, 0:8, TH:T], xt[:, 8:16, 0:TH],
        ]).astype(bf).reshape(4, P, KB1 * TH)  # q11, q22, q12, q21

        W1 = w1[e]
        G1, G2 = W1[0:1024, 0:F], W1[1024:2048, 0:F]
        U1, U2 = W1[0:1024, F:2 * F], W1[1024:2048, F:2 * F]
        s = [G1 + U2, U1 + U2, G1, U2, G1 + G2, U1 - G1, G2 - U2]
        w1s = np.stack([
            sm.reshape(KB1, P, FB, P).transpose(2, 1, 0, 3) for sm in s
        ]).astype(bf)  # [7, 32, 128, 8, 128]

        W2 = w2[e]
        Bp11, Bp12 = W2[0:2048, 0:1024], W2[0:2048, 1024:2048]
        Bp21, Bp22 = W2[2048:4096, 0:1024], W2[2048:4096, 1024:2048]
        r = [Bp11 + Bp22, Bp11, Bp12 - Bp22, Bp21 - Bp11, Bp22,
             Bp11 + Bp12, Bp21 + Bp22]
        w2s = np.stack([
            rm.reshape(KB2, P, HB, P).transpose(2, 1, 0, 3) for rm in r
        ]).astype(bf)  # [7, 8, 128, 16, 128]

        maps.append({
            "xq": np.ascontiguousarray(xq),
            "w1s": np.ascontiguousarray(w1s),
            "w2s": np.ascontiguousarray(w2s),
        })
    return maps


def run_spmd(in_maps, trace=False, trace_kwargs=None):
    from concourse.bass_utils import run_bass_kernel_spmd
    from concourse.bass_interp import get_hw_module

    if "nc" not in _CACHE:
        _CACHE["nc"] = _build()
    nc = _CACHE["nc"]

    old_m = nc.m
    nc.m = get_hw_module(nc.m)
    try:
        res = run_bass_kernel_spmd(
            nc, in_maps, core_ids=list(range(E)),
            trace=trace, **(trace_kwargs or {}),
        )
    finally:
        nc.m = old_m
    return res


def kernel(hidden_states, gate_up_proj, down_proj):
    in_maps = _prep_inputs(hidden_states, gate_up_proj, down_proj)
    res = run_spmd(in_maps)
    # outT [16, 128, 1024]: rows h = hb*128+p, cols t
    out = np.concatenate(
        [res.results[e]["outT"].reshape(H, T).T for e in range(E)], axis=0
    )
    return np.ascontiguousarray(out.astype(np.float32))
